# revision 1
# baseline (speedup 1.0000x reference)
"""MixGARCH Trainium2 kernel.

Reference semantics: scan over t of
    v_t = relu(bias + Wx @ o_t^2 + Wh * v_{t-1}) + 1e-6,  hist[t] = v_t
with bias, Wx, Wh, o^2, v0 all >= 0, so relu is an identity and this is a
LINEAR first-order recurrence:
    v_t = Wh * v_{t-1} + c_t,   c_t = (bias + 1e-6) + Wx @ o_t^2

Strategy (8 cores, full I/O):
 - Each core owns 65536 timesteps, split into 2 halves of 32768. Each half is
   an independent scan lane group (64 components), giving 128 SBUF partitions
   of independent recurrences per core.
 - Cross-boundary state is handled with a 1024-step warmup (Wh < 0.9, so the
   influence of the unknown incoming state decays below fp32 resolution in
   <600 steps; 0.9^1024 ~ 1e-47 == 0.0f). Core 0 half A starts from the exact
   v0 instead (no warmup).
 - On device: PE matmuls compute Wx @ o^2 (zero-padded 32-row weight variants,
   so every matmul is 32-partition aligned), ACT squares the input, copies
   PSUM->SBUF adding (bias + 1e-6) per partition, and DVE tensor_tensor_scan
   runs the recurrence 128 lanes at a time, chained across 512-wide tiles.
 - Host packs the input into the exact SBUF layout (128 = 16 chunks x 8
   channels) and de-interleaves the (128, T'') output back to (T, 64).
"""

import os
import numpy as np

T = 524288
K = 64
NJ = 8
NCORES = 8
W = 1024              # warmup steps per half
HALF = 32768          # real steps per half
TT = W + HALF         # 33792 = per-half scan length
NCH = 8               # chunks per half
CHUNK = TT // NCH     # 4224 elements per partition
F = 512               # scan tile width
NTILES = TT // F      # 66
STAGE = 8 * F         # 4096-wide output staging
MM_DT = os.environ.get("MIXGARCH_MM_DTYPE", "float32")

_CACHE = {}


def _build_nc():
    import concourse.bacc as bacc
    import concourse.mybir as mybir
    import concourse.tile as tile

    mm_dt = getattr(mybir.dt, MM_DT)
    f32 = mybir.dt.float32
    PSUM_BUFS = 6
    CSB_BUFS = 8

    nc = bacc.Bacc(None, target_bir_lowering=False)
    xin = nc.dram_tensor("xin", [128, CHUNK], f32, kind="ExternalInput")
    wt = nc.dram_tensor("wt", [128, 256], f32, kind="ExternalInput")
    biast = nc.dram_tensor("biast", [128, 1], f32, kind="ExternalInput")
    wscan = nc.dram_tensor("wscan", [128, F], f32, kind="ExternalInput")
    vinit = nc.dram_tensor("vinit", [128, 1], f32, kind="ExternalInput")
    vout = nc.dram_tensor("vout", [128, TT], f32, kind="ExternalOutput")

    with tile.TileContext(nc) as tc:
        with (
            tc.tile_pool(name="const", bufs=1) as cpool,
            tc.tile_pool(name="xbuf", bufs=1) as xpool,
            tc.tile_pool(name="cbuf", bufs=1) as cbuf,
            tc.tile_pool(name="stage", bufs=2) as stpool,
            tc.tile_pool(name="psum", bufs=1, space="PSUM") as ps,
        ):
            wt_sb = cpool.tile([128, 256], f32)
            nc.sync.dma_start(wt_sb[:], wt[:])
            bias_sb = cpool.tile([128, 1], f32)
            nc.sync.dma_start(bias_sb[:], biast[:])
            ws_sb = cpool.tile([128, F], f32)
            nc.sync.dma_start(ws_sb[:], wscan[:])
            vi_sb = cpool.tile([128, 1], f32)
            nc.sync.dma_start(vi_sb[:], vinit[:])

            x_sb = xpool.tile([128, CHUNK], f32)
            x2_sb = xpool.tile([128, CHUNK], mm_dt)
            if MM_DT != "float32":
                wt_mm = cpool.tile([128, 256], mm_dt)
                nc.scalar.activation(
                    wt_mm[:], wt_sb[:], mybir.ActivationFunctionType.Copy
                )
            else:
                wt_mm = wt_sb
            NLOAD = 4
            lw = CHUNK // NLOAD  # 1056
            for q in range(NLOAD):
                sl = slice(q * lw, (q + 1) * lw)
                nc.sync.dma_start(x_sb[:, sl], xin[:, sl])
                nc.scalar.activation(
                    x2_sb[:, sl], x_sb[:, sl], mybir.ActivationFunctionType.Square
                )

            prev_stage = None
            stage_t = None
            for i in range(NTILES):
                slot = i % 8
                if slot == 0:
                    prev_stage = stage_t
                    nst = STAGE if (NTILES - i) >= 8 else (NTILES - i) * F
                    stage_t = stpool.tile([128, nst], f32, tag="stage")

                # Per-slot tags pin PSUM reuse to exactly i - PSUM_BUFS.
                c_ps = ps.tile([128, F], f32, tag=f"cps{i % PSUM_BUFS}")
                for h in range(2):
                    done = 0
                    while done < F:
                        pos = i * F + done
                        c = pos // CHUNK
                        off = pos % CHUNK
                        n = min(F - done, CHUNK - off, 512)
                        g = h * NCH + c
                        b, r = g // 4, g % 4
                        lhsT = wt_mm[32 * b:32 * b + 32, 64 * r:64 * r + 64]
                        rhs = x2_sb[32 * b:32 * b + 32, off:off + n]
                        nc.tensor.matmul(
                            c_ps[64 * h:64 * h + 64, done:done + n],
                            lhsT,
                            rhs,
                            start=True,
                            stop=True,
                            tile_position=(32 * b, 64 * h),
                        )
                        done += n

                c_sb = cbuf.tile([128, F], f32, tag=f"csb{i % CSB_BUFS}")
                nc.scalar.activation(
                    c_sb[:], c_ps[:], mybir.ActivationFunctionType.Identity,
                    bias=bias_sb[:, 0:1],
                )

                initial = (
                    vi_sb[:, 0:1]
                    if i == 0
                    else (
                        stage_t[:, slot * F - 1:slot * F]
                        if slot > 0
                        else prev_stage[:, prev_stage.shape[1] - 1:prev_stage.shape[1]]
                    )
                )
                nc.vector.tensor_tensor_scan(
                    stage_t[:, slot * F:(slot + 1) * F],
                    ws_sb[:],
                    c_sb[:],
                    initial,
                    mybir.AluOpType.mult,
                    mybir.AluOpType.add,
                )

                if slot == 7 or i == NTILES - 1:
                    base = (i - slot) * F
                    nc.sync.dma_start(
                        vout[:, base:base + stage_t.shape[1]], stage_t[:]
                    )

    nc.compile()
    return nc


def _host_prep(series, vars0, bias, Wx, Wh):
    series = np.asarray(series, dtype=np.float32)
    vars0 = np.asarray(vars0, dtype=np.float32)
    bias = np.asarray(bias, dtype=np.float32)
    Wx = np.asarray(Wx, dtype=np.float32)
    Wh = np.asarray(Wh, dtype=np.float32)

    in_maps = []
    wt = np.zeros((128, 256), dtype=np.float32)
    for q in range(4):
        for r in range(4):
            for j in range(NJ):
                wt[32 * q + 8 * r + j, 64 * r:64 * r + 64] = Wx[:, j]
    biasv = np.zeros((128, 1), dtype=np.float32)
    biasv[0:64, 0] = bias + 1e-6
    biasv[64:128, 0] = bias + 1e-6
    wscan = np.zeros((128, F), dtype=np.float32)
    wscan[0:64, :] = Wh[:, None]
    wscan[64:128, :] = Wh[:, None]

    for i in range(NCORES):
        xin = np.empty((128, CHUNK), dtype=np.float32)
        for h in range(2):
            start = i * 65536 + h * HALF
            if i == 0 and h == 0:
                rows = series[0:TT]
            else:
                rows = series[start - W:start + HALF]
            for c in range(NCH):
                g = h * NCH + c
                b, r = g // 4, g % 4
                xin[32 * b + 8 * r:32 * b + 8 * r + 8, :] = (
                    rows[c * CHUNK:(c + 1) * CHUNK, :].T
                )
        vinit = np.zeros((128, 1), dtype=np.float32)
        if i == 0:
            vinit[0:64, 0] = vars0
        in_maps.append(
            {"xin": xin, "wt": wt, "biast": biasv, "wscan": wscan, "vinit": vinit}
        )
    return in_maps


def _assemble(results):
    hist = np.empty((T, K), dtype=np.float32)
    for i in range(NCORES):
        vout = results[i]["vout"]
        for h in range(2):
            start = i * 65536 + h * HALF
            q0 = 0 if (i == 0 and h == 0) else W
            hist[start:start + HALF, :] = vout[64 * h:64 * h + 64,
                                               q0:q0 + HALF].T
    return hist


def run(inputs, trace=False, **kw):
    from concourse.bass_utils import run_bass_kernel_spmd

    if "nc" not in _CACHE:
        _CACHE["nc"] = _build_nc()
    nc = _CACHE["nc"]
    in_maps = _host_prep(
        inputs["series"], inputs["vars0"], inputs["bias"],
        inputs["Wx"], inputs["Wh"],
    )
    res = run_bass_kernel_spmd(
        nc, in_maps, core_ids=list(range(NCORES)), trace=trace, **kw
    )
    return _assemble(res.results), res


def kernel(series, vars0, bias, Wx, Wh):
    out, _ = run(
        {"series": series, "vars0": vars0, "bias": bias, "Wx": Wx, "Wh": Wh}
    )
    return out



# revision 11
# speedup vs baseline: 1.4905x; 1.4905x over previous
"""MixGARCH Trainium2 kernel (pair-decimated scan, parity-split input).

Reference: scan over t of v_t = relu(bias + Wx @ o_t^2 + Wh * v_{t-1}) + 1e-6.
All quantities are >= 0 so relu is identity -> LINEAR diagonal recurrence
    v_t = Wh * v_{t-1} + c_t,    c_t = (bias + 1e-6) + Wx @ o_t^2

Per core (8 cores, full I/O): 65536 steps as 2 halves (lanes 64h+k).
Non-initial halves get a 1024-step warmup; core0/half0 uses the exact v0.

Pair decimation: with w_m = v_{2m},
    w_m = Wh^2 w_{m-1} + u_m,     u_m = c_{2m} + Wh c_{2m-1}
    v_{2m+1} = Wh w_m + c_{2m+1}
The DVE scan runs at HALF resolution (u only). Odd steps are reconstructed
and the work is split across engines to balance the pipeline: some tiles via
DVE scalar_tensor_tensor straight from PSUM, the rest via ACT evacuate + ACT
per-partition scale + GPSIMD add.

Input is packed parity-separated: even timesteps in one 32-row block, odd
(shifted by 1) in another, per half -> u is ONE 64-contract matmul per half
(no PSUM accumulation groups, which break codegen here), all rhs contiguous.
Chunks (3 per half) overlay on distinct partition row groups; per-chunk
weight variants select the right rows. Bias rides a ones-channel row.

The linear system is scaled x1024 so outputs fit fp16 normal range: output
DMA is half the bytes. Host interleaves parities and divides by 1024.
"""

import os
import numpy as np

T = 524288
K = 64
NJ = 8
NCORES = 8
W = 1024                # warmup steps per half
HALF = 32768            # real steps per half
TT = W + HALF           # 33792 steps per half-stream
NCH = 3                 # chunks per half
CHUNK = TT // NCH       # 11264 timesteps per chunk
PT = 512                # pairs per tile
PAIRS = TT // 2         # 16896 pairs per half
PPC = CHUNK // 2        # 5632 pairs per chunk
SCOL = PPC + 1          # stored cols per parity row (odd needs +1 lead col)
TPC = PPC // PT         # 11 tiles per chunk
NTILES = NCH * TPC      # 33
SCALE = 1024.0
NLOAD = 4               # xin DMA pieces
DVE_ODD_MOD = int(os.environ.get("MIXGARCH_DVE_ODD_MOD", "3"))

_CACHE = {}


def _build_nc():
    import concourse.bacc as bacc
    import concourse.mybir as mybir
    import concourse.tile as tile

    f32 = mybir.dt.float32
    bf16 = mybir.dt.bfloat16
    fp16 = mybir.dt.float16

    nc = bacc.Bacc(None, target_bir_lowering=False)
    xin = nc.dram_tensor("xin", [128, SCOL], f32, kind="ExternalInput")
    wt = nc.dram_tensor("wt", [128, NCH * 128], f32, kind="ExternalInput")
    whv = nc.dram_tensor("whv", [128, 1], f32, kind="ExternalInput")
    ws2 = nc.dram_tensor("ws2", [128, PT], f32, kind="ExternalInput")
    vinit = nc.dram_tensor("vinit", [128, 1], f32, kind="ExternalInput")
    voutE = nc.dram_tensor("voutE", [128, PAIRS], fp16, kind="ExternalOutput")
    voutO = nc.dram_tensor("voutO", [128, PAIRS], fp16, kind="ExternalOutput")

    with tile.TileContext(nc) as tc:
        with (
            tc.tile_pool(name="const", bufs=1) as cpool,
            tc.tile_pool(name="xload", bufs=1) as xl,
            tc.tile_pool(name="x2buf", bufs=1) as xp,
            tc.tile_pool(name="csb", bufs=1) as cp,
            tc.tile_pool(name="tmp", bufs=1) as tp,
            tc.tile_pool(name="stgE", bufs=2) as se,
            tc.tile_pool(name="stgO", bufs=2) as so,
            tc.tile_pool(name="psum", bufs=1, space="PSUM") as ps,
        ):
            wt_f = cpool.tile([128, NCH * 128], f32)
            nc.sync.dma_start(wt_f[:], wt[:])
            wt_sb = cpool.tile([128, NCH * 128], bf16)
            nc.scalar.activation(wt_sb[:], wt_f[:],
                                 mybir.ActivationFunctionType.Copy)
            whv_sb = cpool.tile([128, 1], f32)
            nc.sync.dma_start(whv_sb[:], whv[:])
            ws2_sb = cpool.tile([128, PT], f32)
            nc.sync.dma_start(ws2_sb[:], ws2[:])
            vi_sb = cpool.tile([128, 1], f32)
            nc.sync.dma_start(vi_sb[:], vinit[:])

            # x2: squared input, bf16, rows = [h0-even | h0-odd | h1-even |
            # h1-odd] 32-row blocks, each holding 3 chunk groups + ones row.
            x2_sb = xp.tile([128, SCOL], bf16)
            piece = (SCOL + NLOAD - 1) // NLOAD
            for q in range(NLOAD):
                a = q * piece
                b = min(SCOL, a + piece)
                xs = xl.tile([128, b - a], f32, tag=f"x{q % 2}")
                nc.sync.dma_start(xs[:], xin[:, a:b])
                nc.scalar.activation(x2_sb[:, a:b], xs[:],
                                     mybir.ActivationFunctionType.Square)

            stage_e = None
            stage_o = None
            prev_e = None
            for s in range(NTILES):
                ch = s // TPC
                slot = s % TPC
                if slot == 0:
                    prev_e = stage_e
                    stage_e = se.tile([128, PPC], fp16, tag="se")
                    stage_o = so.tile([128, PPC], fp16, tag="so")

                m0 = slot * PT
                pu = ps.tile([128, PT], f32, tag=f"u{s % 3}")
                pc = ps.tile([128, PT], f32, tag=f"c{s % 3}")
                for h in range(2):
                    outp = slice(64 * h, 64 * h + 64)
                    # u_m for pairs [m0, m0+PT): one 64-contract matmul
                    nc.tensor.matmul(
                        pu[outp, :],
                        wt_sb[64 * h:64 * h + 64, 128 * ch:128 * ch + 64],
                        x2_sb[64 * h:64 * h + 64, m0:m0 + PT],
                        start=True, stop=True, tile_position=(64 * h, 64 * h),
                    )
                    # c_{2m+1}: odd block only, shifted one col
                    nc.tensor.matmul(
                        pc[outp, :],
                        wt_sb[64 * h + 32:64 * h + 64,
                              128 * ch + 64:128 * ch + 128],
                        x2_sb[64 * h + 32:64 * h + 64, m0 + 1:m0 + PT + 1],
                        start=True, stop=True,
                        tile_position=(64 * h + 32, 64 * h),
                    )

                initial = (
                    vi_sb[:, 0:1] if s == 0
                    else (stage_e[:, m0 - 1:m0] if slot > 0
                          else prev_e[:, PPC - 1:PPC])
                )
                nc.vector.tensor_tensor_scan(
                    stage_e[:, m0:m0 + PT], ws2_sb[:], pu[:], initial,
                    mybir.AluOpType.mult, mybir.AluOpType.add,
                )

                if s % DVE_ODD_MOD == 0:
                    # odd path A: fused on DVE straight from PSUM
                    nc.vector.scalar_tensor_tensor(
                        stage_o[:, m0:m0 + PT], stage_e[:, m0:m0 + PT],
                        whv_sb[:, 0:1], pc[:],
                        mybir.AluOpType.mult, mybir.AluOpType.add,
                    )
                else:
                    # odd path B: ACT evacuate + ACT scale + GPSIMD add
                    csb = cp.tile([128, PT], f32, tag=f"cs{s % 4}")
                    nc.scalar.activation(
                        csb[:], pc[:], mybir.ActivationFunctionType.Copy
                    )
                    tmp = tp.tile([128, PT], fp16, tag=f"t{s % 4}")
                    nc.scalar.activation(
                        tmp[:], stage_e[:, m0:m0 + PT],
                        mybir.ActivationFunctionType.Copy,
                        scale=whv_sb[:, 0:1],
                    )
                    nc.gpsimd.tensor_tensor(
                        stage_o[:, m0:m0 + PT], tmp[:], csb[:],
                        mybir.AluOpType.add,
                    )

                if slot == TPC - 1:
                    nc.sync.dma_start(
                        voutE[:, ch * PPC:(ch + 1) * PPC], stage_e[:]
                    )
                    nc.sync.dma_start(
                        voutO[:, ch * PPC:(ch + 1) * PPC], stage_o[:]
                    )

    nc.compile()
    return nc


def _host_prep(series, vars0, bias, Wx, Wh):
    series = np.asarray(series, dtype=np.float32)
    vars0 = np.asarray(vars0, dtype=np.float64)
    bias = np.asarray(bias, dtype=np.float64)
    Wx = np.asarray(Wx, dtype=np.float64)
    Wh = np.asarray(Wh, dtype=np.float64)

    biasp = bias + 1e-6
    # wt free layout per chunk ch: [Wu (64 wide) | Wc (64 wide)]
    # Wu rows (partitions 0-63): even part at 8ch+i, ones at 24;
    #                            odd part (Wh-scaled) at 32+8ch+i, ones at 56.
    # Wc rows (partitions 32-63): 8ch+i and ones at 24 within the block.
    # Rows 64-127 duplicate 0-63 for the h=1 tile positions.
    wt = np.zeros((128, NCH * 128), dtype=np.float64)
    for c in range(NCH):
        u0 = 128 * c
        c0 = 128 * c + 64
        for j in range(NJ):
            wt[8 * c + j, u0:u0 + 64] = SCALE * Wx[:, j]
            wt[32 + 8 * c + j, u0:u0 + 64] = SCALE * Wx[:, j] * Wh
            wt[32 + 8 * c + j, c0:c0 + 64] = SCALE * Wx[:, j]
        wt[24, u0:u0 + 64] = SCALE * biasp
        wt[56, u0:u0 + 64] = SCALE * biasp * Wh
        wt[56, c0:c0 + 64] = SCALE * biasp
    wt[64:128, :] = wt[0:64, :]
    wt = wt.astype(np.float32)

    whv = np.tile(Wh, 2).reshape(128, 1).astype(np.float32)
    ws2 = np.repeat(
        np.tile(Wh * Wh, 2).reshape(128, 1), PT, axis=1
    ).astype(np.float32)

    in_maps = []
    for i in range(NCORES):
        xin = np.zeros((128, SCOL), dtype=np.float32)
        vinit = np.zeros((128, 1), dtype=np.float32)
        for h in range(2):
            t0 = i * 65536 + h * HALF
            first = i == 0 and h == 0
            s0 = t0 - (0 if first else W)
            for c in range(NCH):
                tc0 = s0 + c * CHUNK
                er = 64 * h + 8 * c
                orr = 64 * h + 32 + 8 * c
                # even cols j=0..PPC-1: series[tc0 + 2j]
                xin[er:er + 8, 0:PPC] = series[tc0:tc0 + CHUNK:2, :].T
                # odd cols j=0..PPC: series[tc0 + 2j - 1]
                start = tc0 - 1
                j0 = 0 if start >= 0 else (1 - start) // 2
                ov = np.zeros((SCOL, NJ), dtype=np.float32)
                ov[j0:, :] = series[start + 2 * j0:start + 2 * SCOL:2, :]
                xin[orr:orr + 8, :] = ov.T
            xin[64 * h + 24, :] = 1.0
            xin[64 * h + 56, :] = 1.0
        if i == 0:
            w0 = SCALE * (vars0 - biasp) / np.maximum(Wh, 1e-20)
            w0 = np.where(Wh < 1e-20, 0.0, w0)
            vinit[0:64, 0] = w0.astype(np.float32)
        in_maps.append({
            "xin": xin, "wt": wt, "whv": whv, "ws2": ws2, "vinit": vinit,
        })
    return in_maps


def _assemble(results):
    hist = np.empty((T, K), dtype=np.float32)
    inv = np.float32(1.0 / SCALE)
    for i in range(NCORES):
        vE = results[i]["voutE"]
        vO = results[i]["voutO"]
        for h in range(2):
            t0 = i * 65536 + h * HALF
            q0 = 0 if (i == 0 and h == 0) else W // 2
            e = vE[64 * h:64 * h + 64, q0:q0 + HALF // 2].astype(np.float32)
            o = vO[64 * h:64 * h + 64, q0:q0 + HALF // 2].astype(np.float32)
            blk = hist[t0:t0 + HALF, :]
            blk[0::2, :] = e.T * inv
            blk[1::2, :] = o.T * inv
    return hist


def run(inputs, trace=False, **kw):
    from concourse.bass_utils import run_bass_kernel_spmd

    if "nc" not in _CACHE:
        _CACHE["nc"] = _build_nc()
    nc = _CACHE["nc"]
    in_maps = _host_prep(
        inputs["series"], inputs["vars0"], inputs["bias"],
        inputs["Wx"], inputs["Wh"],
    )
    res = run_bass_kernel_spmd(
        nc, in_maps, core_ids=list(range(NCORES)), trace=trace, **kw
    )
    return _assemble(res.results), res


def kernel(series, vars0, bias, Wx, Wh):
    out, _ = run(
        {"series": series, "vars0": vars0, "bias": bias, "Wx": Wx, "Wh": Wh}
    )
    return out


# revision 12
# speedup vs baseline: 1.6423x; 1.1018x over previous
"""MixGARCH Trainium2 kernel (pair-decimated scan, parity-split input).

Reference: scan over t of v_t = relu(bias + Wx @ o_t^2 + Wh * v_{t-1}) + 1e-6.
All quantities are >= 0 so relu is identity -> LINEAR diagonal recurrence
    v_t = Wh * v_{t-1} + c_t,    c_t = (bias + 1e-6) + Wx @ o_t^2

Per core (8 cores, full I/O): 65536 steps as 2 halves (lanes 64h+k).
Non-initial halves get a 1024-step warmup; core0/half0 uses the exact v0.

Pair decimation: with w_m = v_{2m},
    w_m = Wh^2 w_{m-1} + u_m,     u_m = c_{2m} + Wh c_{2m-1}
    v_{2m+1} = Wh w_m + c_{2m+1}
The DVE scan runs at HALF resolution (u only). Odd steps are reconstructed
and the work is split across engines to balance the pipeline: some tiles via
DVE scalar_tensor_tensor straight from PSUM, the rest via ACT evacuate + ACT
per-partition scale + GPSIMD add.

Input is packed parity-separated: even timesteps in one 32-row block, odd
(shifted by 1) in another, per half -> u is ONE 64-contract matmul per half
(no PSUM accumulation groups, which break codegen here), all rhs contiguous.
Chunks (3 per half) overlay on distinct partition row groups; per-chunk
weight variants select the right rows. Bias rides a ones-channel row.

The linear system is scaled x1024 so outputs fit fp16 normal range: output
DMA is half the bytes. Host interleaves parities and divides by 1024.
"""

import os
import numpy as np

T = 524288
K = 64
NJ = 8
NCORES = 8
W = 1024                # warmup steps per half
HALF = 32768            # real steps per half
TT = W + HALF           # 33792 steps per half-stream
NCH = 3                 # chunks per half
CHUNK = TT // NCH       # 11264 timesteps per chunk
PT = 512                # pairs per tile
PAIRS = TT // 2         # 16896 pairs per half
PPC = CHUNK // 2        # 5632 pairs per chunk
SCOL = PPC + 1          # stored cols per parity row (odd needs +1 lead col)
TPC = PPC // PT         # 11 tiles per chunk
NTILES = NCH * TPC      # 33
SCALE = 1024.0
NLOAD = 4               # xin DMA pieces
DVE_ODD_MOD = int(os.environ.get("MIXGARCH_DVE_ODD_MOD", "4"))

_CACHE = {}


def _build_nc():
    import concourse.bacc as bacc
    import concourse.mybir as mybir
    import concourse.tile as tile

    f32 = mybir.dt.float32
    bf16 = mybir.dt.bfloat16
    fp16 = mybir.dt.float16

    nc = bacc.Bacc(None, target_bir_lowering=False)
    xin = nc.dram_tensor("xin", [128, SCOL], f32, kind="ExternalInput")
    wt = nc.dram_tensor("wt", [128, NCH * 256], f32, kind="ExternalInput")
    whv = nc.dram_tensor("whv", [128, 1], f32, kind="ExternalInput")
    ws2 = nc.dram_tensor("ws2", [128, PT], f32, kind="ExternalInput")
    vinit = nc.dram_tensor("vinit", [128, 1], f32, kind="ExternalInput")
    voutE = nc.dram_tensor("voutE", [128, PAIRS], fp16, kind="ExternalOutput")
    voutO = nc.dram_tensor("voutO", [128, PAIRS], fp16, kind="ExternalOutput")

    with tile.TileContext(nc) as tc:
        with (
            tc.tile_pool(name="const", bufs=1) as cpool,
            tc.tile_pool(name="xload", bufs=1) as xl,
            tc.tile_pool(name="x2buf", bufs=1) as xp,
            tc.tile_pool(name="csb", bufs=1) as cp,
            tc.tile_pool(name="tmp", bufs=1) as tp,
            tc.tile_pool(name="stgE", bufs=2) as se,
            tc.tile_pool(name="stgO", bufs=2) as so,
            tc.tile_pool(name="psum", bufs=1, space="PSUM") as ps,
        ):
            wt_f = cpool.tile([128, NCH * 256], f32)
            nc.sync.dma_start(wt_f[:], wt[:])
            wt_sb = cpool.tile([128, NCH * 256], bf16)
            nc.scalar.activation(wt_sb[:], wt_f[:],
                                 mybir.ActivationFunctionType.Copy)
            whv_sb = cpool.tile([128, 1], f32)
            nc.sync.dma_start(whv_sb[:], whv[:])
            ws2_sb = cpool.tile([128, PT], f32)
            nc.sync.dma_start(ws2_sb[:], ws2[:])
            vi_sb = cpool.tile([128, 1], f32)
            nc.sync.dma_start(vi_sb[:], vinit[:])

            # x2: squared input, bf16, rows = [h0-even | h0-odd | h1-even |
            # h1-odd] 32-row blocks, each holding 3 chunk groups + ones row.
            x2_sb = xp.tile([128, SCOL], bf16)
            piece = (SCOL + NLOAD - 1) // NLOAD
            for q in range(NLOAD):
                a = q * piece
                b = min(SCOL, a + piece)
                xs = xl.tile([128, b - a], f32, tag=f"x{q % 2}")
                nc.sync.dma_start(xs[:], xin[:, a:b])
                nc.scalar.activation(x2_sb[:, a:b], xs[:],
                                     mybir.ActivationFunctionType.Square)

            stage_e = None
            stage_o = None
            prev_e = None
            for s in range(NTILES):
                ch = s // TPC
                slot = s % TPC
                if slot == 0:
                    prev_e = stage_e
                    stage_e = se.tile([128, PPC], fp16, tag="se")
                    stage_o = so.tile([128, PPC], fp16, tag="so")

                m0 = slot * PT
                pu = ps.tile([128, PT], f32, tag=f"u{s % 3}")
                pc = ps.tile([128, PT], f32, tag=f"c{s % 3}")
                # block-diagonal 128-contract matmuls: both halves at once
                nc.tensor.matmul(
                    pu[:, :], wt_sb[:, 256 * ch:256 * ch + 128],
                    x2_sb[:, m0:m0 + PT],
                    start=True, stop=True, tile_position=(0, 0),
                )
                nc.tensor.matmul(
                    pc[:, :], wt_sb[:, 256 * ch + 128:256 * ch + 256],
                    x2_sb[:, m0 + 1:m0 + PT + 1],
                    start=True, stop=True, tile_position=(0, 0),
                )

                initial = (
                    vi_sb[:, 0:1] if s == 0
                    else (stage_e[:, m0 - 1:m0] if slot > 0
                          else prev_e[:, PPC - 1:PPC])
                )
                nc.vector.tensor_tensor_scan(
                    stage_e[:, m0:m0 + PT], ws2_sb[:], pu[:], initial,
                    mybir.AluOpType.mult, mybir.AluOpType.add,
                )

                if s % DVE_ODD_MOD == 0:
                    # odd path A: fused on DVE straight from PSUM
                    nc.vector.scalar_tensor_tensor(
                        stage_o[:, m0:m0 + PT], stage_e[:, m0:m0 + PT],
                        whv_sb[:, 0:1], pc[:],
                        mybir.AluOpType.mult, mybir.AluOpType.add,
                    )
                else:
                    # odd path B: ACT evacuate + ACT scale + GPSIMD add
                    csb = cp.tile([128, PT], f32, tag=f"cs{s % 4}")
                    nc.scalar.activation(
                        csb[:], pc[:], mybir.ActivationFunctionType.Copy
                    )
                    tmp = tp.tile([128, PT], fp16, tag=f"t{s % 4}")
                    nc.scalar.activation(
                        tmp[:], stage_e[:, m0:m0 + PT],
                        mybir.ActivationFunctionType.Copy,
                        scale=whv_sb[:, 0:1],
                    )
                    nc.gpsimd.tensor_tensor(
                        stage_o[:, m0:m0 + PT], tmp[:], csb[:],
                        mybir.AluOpType.add,
                    )

                if slot == TPC - 1:
                    nc.sync.dma_start(
                        voutE[:, ch * PPC:(ch + 1) * PPC], stage_e[:]
                    )
                    nc.sync.dma_start(
                        voutO[:, ch * PPC:(ch + 1) * PPC], stage_o[:]
                    )

    nc.compile()
    return nc


def _host_prep(series, vars0, bias, Wx, Wh):
    series = np.asarray(series, dtype=np.float32)
    vars0 = np.asarray(vars0, dtype=np.float64)
    bias = np.asarray(bias, dtype=np.float64)
    Wx = np.asarray(Wx, dtype=np.float64)
    Wh = np.asarray(Wh, dtype=np.float64)

    biasp = bias + 1e-6
    # wt free layout per chunk ch: [Wu-big (128 wide) | Wc-big (128 wide)],
    # both block-diagonal over the two halves (rows 64h+* -> cols 64h+*).
    wt = np.zeros((128, NCH * 256), dtype=np.float64)
    for c in range(NCH):
        for h in range(2):
            u0 = 256 * c + 64 * h
            c0 = 256 * c + 128 + 64 * h
            r = 64 * h
            for j in range(NJ):
                wt[r + 8 * c + j, u0:u0 + 64] = SCALE * Wx[:, j]
                wt[r + 32 + 8 * c + j, u0:u0 + 64] = SCALE * Wx[:, j] * Wh
                wt[r + 32 + 8 * c + j, c0:c0 + 64] = SCALE * Wx[:, j]
            wt[r + 24, u0:u0 + 64] = SCALE * biasp
            wt[r + 56, u0:u0 + 64] = SCALE * biasp * Wh
            wt[r + 56, c0:c0 + 64] = SCALE * biasp
    wt = wt.astype(np.float32)

    whv = np.tile(Wh, 2).reshape(128, 1).astype(np.float32)
    ws2 = np.repeat(
        np.tile(Wh * Wh, 2).reshape(128, 1), PT, axis=1
    ).astype(np.float32)

    in_maps = []
    for i in range(NCORES):
        xin = np.zeros((128, SCOL), dtype=np.float32)
        vinit = np.zeros((128, 1), dtype=np.float32)
        for h in range(2):
            t0 = i * 65536 + h * HALF
            first = i == 0 and h == 0
            s0 = t0 - (0 if first else W)
            for c in range(NCH):
                tc0 = s0 + c * CHUNK
                er = 64 * h + 8 * c
                orr = 64 * h + 32 + 8 * c
                # even cols j=0..PPC-1: series[tc0 + 2j]
                xin[er:er + 8, 0:PPC] = series[tc0:tc0 + CHUNK:2, :].T
                # odd cols j=0..PPC: series[tc0 + 2j - 1]
                start = tc0 - 1
                j0 = 0 if start >= 0 else (1 - start) // 2
                ov = np.zeros((SCOL, NJ), dtype=np.float32)
                ov[j0:, :] = series[start + 2 * j0:start + 2 * SCOL:2, :]
                xin[orr:orr + 8, :] = ov.T
            xin[64 * h + 24, :] = 1.0
            xin[64 * h + 56, :] = 1.0
        if i == 0:
            w0 = SCALE * (vars0 - biasp) / np.maximum(Wh, 1e-20)
            w0 = np.where(Wh < 1e-20, 0.0, w0)
            vinit[0:64, 0] = w0.astype(np.float32)
        in_maps.append({
            "xin": xin, "wt": wt, "whv": whv, "ws2": ws2, "vinit": vinit,
        })
    return in_maps


def _assemble(results):
    hist = np.empty((T, K), dtype=np.float32)
    inv = np.float32(1.0 / SCALE)
    for i in range(NCORES):
        vE = results[i]["voutE"]
        vO = results[i]["voutO"]
        for h in range(2):
            t0 = i * 65536 + h * HALF
            q0 = 0 if (i == 0 and h == 0) else W // 2
            e = vE[64 * h:64 * h + 64, q0:q0 + HALF // 2].astype(np.float32)
            o = vO[64 * h:64 * h + 64, q0:q0 + HALF // 2].astype(np.float32)
            blk = hist[t0:t0 + HALF, :]
            blk[0::2, :] = e.T * inv
            blk[1::2, :] = o.T * inv
    return hist


def run(inputs, trace=False, **kw):
    from concourse.bass_utils import run_bass_kernel_spmd

    if "nc" not in _CACHE:
        _CACHE["nc"] = _build_nc()
    nc = _CACHE["nc"]
    in_maps = _host_prep(
        inputs["series"], inputs["vars0"], inputs["bias"],
        inputs["Wx"], inputs["Wh"],
    )
    res = run_bass_kernel_spmd(
        nc, in_maps, core_ids=list(range(NCORES)), trace=trace, **kw
    )
    return _assemble(res.results), res


def kernel(series, vars0, bias, Wx, Wh):
    out, _ = run(
        {"series": series, "vars0": vars0, "bias": bias, "Wx": Wx, "Wh": Wh}
    )
    return out
